# revision 4
# baseline (speedup 1.0000x reference)
"""Trainium2 Bass kernel for nn_LocalLoadBalancingLoss.

loss = mean_b var_l(u) + 0.5 * mean_b max_l(u),
u[b,l] = (sum_{t: link(t)=l} pred[b,t] * dem[b, t//8]) / (cap[l] + 1e-8)

Pure data parallel over batch: 8 cores x 8192 rows, scalar partials
combined on host. Measured ~85us/pass/core vs a ~82.6us pure-DMA floor
(memory-bound; ~3.6x faster than the fp32 transpose+matmul baseline's
~307us).

Design:
  Row mapping per core: tile k holds rows {p*64 + k : p in 0..127} so dem
  loads contiguously per partition and adjacent tiles pair into single
  6336B-per-line DMAs (the loss is row-permutation invariant, so any
  row->tile mapping works).

  per PAIR of 128-row tiles (32 pairs per core):
    - DMA pred [128,2,792] on SP or Pool queue (spread over 2 DGE queues)
    - DVE: tt = pred * broadcast(dem)  (fp32 in -> bf16 out, one op/pair)
    - PE : 14x bf16 transpose of tt chunks -> PSUM (identity stays loaded
           across a whole group of transposes)
    - ACT: evacuate ttT chunks into a per-group SBUF buffer [P,7,G,P]
  per group of G=4 tiles:
    - PE : 7 scatter matmuls, mask chunk [128,16] bf16 STATIONARY (1/cap
           folded into the mask on host), rhs streams all 4 tiles at once
           -> uT_ps [16, 512] f32 accumulated in one PSUM bank
    - ACT: evacuate uT to SBUF
    - PE : 4 back-transposes uT -> u [128, G, 16] f32 PSUM
  per super-group of 16 tiles:
    - DVE/ACT: fused stats (row-sum, row-max, sum of squares) into
      [128, nsg] accumulators
  Host: final tiny reduction + combine across 8 cores.
"""

from contextlib import ExitStack

import numpy as np

import concourse.bass as bass
import concourse.tile as tile
from concourse import mybir
from concourse.bass_utils import run_bass_kernel_spmd
from bass_rust import ScopedClock

N_CORES = 8
B, T, D, L = 65536, 792, 99, 16
ROWS = B // N_CORES  # 8192 rows per core
P = 128
NT = ROWS // P  # 64 tiles per core
NCH = (T + P - 1) // P  # 7 tunnel chunks
TPAD = NCH * P  # 896 (792 zero-padded)
G = 4  # tiles per group (G*P f32 = 2KB = one PSUM bank, matmul can't cross)
NG = NT // G
SGG = 4  # groups per stats super-group (amortizes DVE PSUM-access init)
DEMCH = 4  # dem arrives in 4 chunks of NT/4 tiles each

F32 = mybir.dt.float32
BF16 = mybir.dt.bfloat16
X = mybir.AxisListType


class _TileContext(tile.TileContext):
    """Workaround: this walrus build allows only 1 sync-wait per
    instruction; stock TileContext packs one wait per outstanding proc
    onto the single tail drain. Spread them across multiple drains."""

    def _drain_and_barrier(self, tick_clock, wait_clock):
        nc = self.nc
        drain_inst = nc.sync.drain()
        wait_clock.add_sem_waits(
            drain_inst.ins, ScopedClock({None: tick_clock.global_clock})
        )
        si = drain_inst.ins.sync_info
        waits = list(si.on_wait) if si is not None and si.on_wait else []
        if len(waits) > 1:
            drain_inst.ins.sync_info = mybir.SyncInfo(
                on_wait=[waits[0]], on_update=list(si.on_update or [])
            )
            for w in waits[1:]:
                d = nc.sync.drain()
                d.ins.sync_info = mybir.SyncInfo(on_wait=[w], on_update=[])
        nc.all_engine_barrier()
        assert self.sems is not None
        popped = nc._tile_sem_poison_stack.pop()
        assert popped is self._sem_poison
        nc.clear_and_free_semaphores(list(self.sems.allocated().values()))
        nc.all_engine_barrier()


def _split_multi_waits(nc):
    """This walrus build accepts only 1 sync-wait per instruction (2 for
    EventSemaphore). Hoist extra semaphore waits onto same-engine NOPs
    inserted immediately before the instruction (engine queues are strict
    FIFO, so a preceding wait-NOP is semantically identical)."""
    for fn in nc.m.functions:
        for blk in fn.blocks:
            insts = blk.instructions
            out = []
            for inst in insts:
                si = inst.sync_info
                waits = list(si.on_wait) if si is not None and si.on_wait else []
                cap = 2 if isinstance(inst, mybir.InstEventSemaphore) else 1
                if len(waits) > cap and inst.engine != mybir.EngineType.Unassigned:
                    for w in waits[:-1]:
                        nop = mybir.InstNoOp(
                            name=f"{inst.name}-w{len(out)}",
                            engine=inst.engine,
                            sync_info=mybir.SyncInfo(on_wait=[w], on_update=[]),
                            bass_nofuse=True,
                        )
                        nc.register_instruction(nop, overwrite=True)
                        out.append(nop)
                    inst.sync_info = mybir.SyncInfo(
                        on_wait=[waits[-1]], on_update=list(si.on_update or [])
                    )
                out.append(inst)
            blk.instructions = out


def build_kernel(repeat=1, n_tiles=NT):
    ng = max(1, n_tiles // G)
    sgg = min(SGG, ng)  # groups per super-group
    nsg = ng // sgg
    nc = bass.Bass("TRN2", target_bir_lowering=False, debug=False, num_devices=1)
    pred_d = nc.dram_tensor("pred", [ROWS, T], F32, kind="ExternalInput")
    dem_d = nc.dram_tensor("dem", [ROWS, D], F32, kind="ExternalInput")
    mask_d = nc.dram_tensor("mask", [P, NCH * L], BF16, kind="ExternalInput")
    identb_d = nc.dram_tensor("identb", [P, P], BF16, kind="ExternalInput")
    identf_d = nc.dram_tensor("identf", [L, L], F32, kind="ExternalInput")
    out_d = nc.dram_tensor("partials", [3, P, nsg], F32, kind="ExternalOutput")

    pred_v = pred_d.ap().rearrange("(p k) t -> p k t", p=P)  # row = p*NT + k
    dem_v = dem_d.ap().rearrange("(p k) d -> p k d", p=P)

    with _TileContext(nc) as tc:
        with ExitStack() as ctx:
            singles = ctx.enter_context(tc.tile_pool(name="singles", bufs=1))
            io = ctx.enter_context(tc.tile_pool(name="io", bufs=6))
            work = ctx.enter_context(tc.tile_pool(name="work", bufs=4))
            tsb = ctx.enter_context(tc.tile_pool(name="tsb", bufs=2))
            usb = ctx.enter_context(tc.tile_pool(name="usb", bufs=2))
            small = ctx.enter_context(tc.tile_pool(name="small", bufs=2))
            sq = ctx.enter_context(tc.tile_pool(name="sq", bufs=2))
            tpsum = ctx.enter_context(tc.tile_pool(name="tpsum", bufs=2, space="PSUM"))
            upsum = ctx.enter_context(tc.tile_pool(name="upsum", bufs=2, space="PSUM"))
            u2psum = ctx.enter_context(
                tc.tile_pool(name="u2psum", bufs=2, space="PSUM")
            )

            # constants ride the otherwise-idle ACT queue at startup
            identb_t = singles.tile([P, P], BF16)
            nc.scalar.dma_start(identb_t[:], identb_d.ap())
            identf_t = singles.tile([L, L], F32)
            nc.scalar.dma_start(identf_t[:], identf_d.ap())
            mask_t = singles.tile([P, NCH * L], BF16)
            nc.scalar.dma_start(mask_t[:], mask_d.ap())
            # dem arrives in per-super-group chunks; chunk 0 up front, the
            # rest staggered one super-group ahead of use
            dem_all = singles.tile([P, NT, D], F32)
            dck = sgg * G  # one super-group's worth of rows per chunk

            def load_dem_chunk(dc):
                if dc < n_tiles:
                    nc.sync.dma_start(
                        dem_all[:, dc : dc + dck, :], dem_v[:, dc : dc + dck, :]
                    )

            accq = singles.tile([P, nsg], F32)
            accs2 = singles.tile([P, nsg], F32)
            accm = singles.tile([P, nsg], F32)

            for rep in range(repeat):
                # dem reloads every pass so repeat-timing counts its traffic
                if n_tiles > dck // 2:
                    nc.sync.dma_start(
                        dem_all[:, 0 : dck // 2, :], dem_v[:, 0 : dck // 2, :]
                    )
                    nc.sync.dma_start(
                        dem_all[:, dck // 2 : dck, :], dem_v[:, dck // 2 : dck, :]
                    )
                else:
                    load_dem_chunk(0)
                for sg in range(nsg):
                    load_dem_chunk((sg + 1) * dck)
                    u2 = u2psum.tile([P, sgg, G, L], F32)
                    for gg in range(sgg):
                        g = sg * sgg + gg
                        ttT_g = tsb.tile([P, NCH, G, P], BF16)
                        for jp in range(G // 2):
                            pair = g * (G // 2) + jp
                            k = (2 * pair) % n_tiles
                            pred_t = io.tile([P, 2, T], F32)
                            # spread pred loads over two DMA queues: SP
                            # (HWDGE) and Pool (SWDGE, ~1us/issue, idle)
                            dma_eng = nc.gpsimd if pair % 4 == 1 else nc.sync
                            dma_eng.dma_start(pred_t[:], pred_v[:, k : k + 2, :])

                            tt = work.tile([P, 2, TPAD], BF16)
                            nc.gpsimd.memset(tt[:, :, T:TPAD], 0.0)
                            nc.vector.tensor_tensor(
                                out=tt[:, :, 0:T].rearrange(
                                    "p i (d j) -> p i d j", j=8
                                ),
                                in0=pred_t[:].rearrange("p i (d j) -> p i d j", j=8),
                                in1=dem_all[:, k : k + 2, :]
                                .unsqueeze(3)
                                .broadcast_to([P, 2, D, 8]),
                                op=mybir.AluOpType.mult,
                            )
                            ttT_ps = tpsum.tile([P, 2, TPAD], BF16)
                            for i in range(2):
                                for c in range(NCH):
                                    nc.tensor.transpose(
                                        out=ttT_ps[:, i, c * P : (c + 1) * P],
                                        in_=tt[:, i, c * P : (c + 1) * P],
                                        identity=identb_t[:],
                                    )
                            nc.scalar.copy(
                                out=ttT_g[:, :, 2 * jp : 2 * jp + 2, :],
                                in_=ttT_ps[:].rearrange("p i (c r) -> p c i r", c=NCH),
                            )
                        # --- scatter: uT[l,b] += mask_c^T @ ttT_c, G tiles ---
                        uT_ps = upsum.tile([L, G * P], F32)
                        for c in range(NCH):
                            nc.tensor.matmul(
                                out=uT_ps[:],
                                lhsT=mask_t[:, c * L : (c + 1) * L],
                                rhs=ttT_g[:, c, :, :].rearrange("p g r -> p (g r)"),
                                start=(c == 0),
                                stop=(c == NCH - 1),
                            )
                        uT_sb = usb.tile([L, G * P], F32)
                        nc.scalar.copy(uT_sb[:], uT_ps[:])
                        # --- back-transpose to natural layout [b, l] ---
                        for j in range(G):
                            nc.tensor.transpose(
                                out=u2[:, gg, j, :],
                                in_=uT_sb[:, j * P : (j + 1) * P],
                                identity=identf_t[:],
                            )
                    # --- stats for this super-group of sgg*G tiles ---
                    s8 = small.tile([P, sgg * G], F32)
                    nc.vector.reduce_sum(
                        out=s8[:], in_=u2[:].rearrange("p a g l -> p (a g) l"), axis=X.X
                    )
                    m8 = small.tile([P, sgg * G], F32)
                    nc.vector.reduce_max(
                        out=m8[:], in_=u2[:].rearrange("p a g l -> p (a g) l"), axis=X.X
                    )
                    usq = sq.tile([P, sgg * G, L], F32)
                    nc.scalar.square(
                        out=usq[:],
                        in_=u2[:].rearrange("p a g l -> p (a g) l"),
                    )
                    nc.vector.reduce_sum(
                        out=accq[:, sg : sg + 1], in_=usq[:], axis=X.XY
                    )
                    s2s = small.tile([P, sgg * G], F32)
                    nc.gpsimd.tensor_tensor(
                        out=s2s[:], in0=s8[:], in1=s8[:], op=mybir.AluOpType.mult
                    )
                    nc.vector.reduce_sum(
                        out=accs2[:, sg : sg + 1], in_=s2s[:], axis=X.X
                    )
                    nc.vector.reduce_sum(out=accm[:, sg : sg + 1], in_=m8[:], axis=X.X)
            nc.sync.dma_start(out_d.ap()[0], accq[:])
            nc.sync.dma_start(out_d.ap()[1], accs2[:])
            nc.sync.dma_start(out_d.ap()[2], accm[:])
    _split_multi_waits(nc)
    return nc


def make_constants(tunnel_to_link, link_capacities):
    import ml_dtypes

    t2l = np.asarray(tunnel_to_link).astype(np.int64).ravel()
    cap = np.asarray(link_capacities, dtype=np.float32).ravel()
    mask = np.zeros((P, NCH * L), dtype=np.float32)
    rcap = 1.0 / (cap + 1e-8)
    for t in range(T):
        c, r = divmod(t, P)
        mask[r, c * L + int(t2l[t])] = rcap[int(t2l[t])]
    identb = np.eye(P, dtype=np.float32)
    identf = np.eye(L, dtype=np.float32)
    return (
        mask.astype(ml_dtypes.bfloat16),
        identb.astype(ml_dtypes.bfloat16),
        identf,
    )


def run_cores(nc, pred, dem, mask, identb, identf, **kw):
    pred = np.ascontiguousarray(np.asarray(pred, dtype=np.float32))
    dem = np.ascontiguousarray(np.asarray(dem, dtype=np.float32))
    in_maps = []
    for i in range(N_CORES):
        in_maps.append(
            {
                "pred": pred[i * ROWS : (i + 1) * ROWS],
                "dem": dem[i * ROWS : (i + 1) * ROWS],
                "mask": mask,
                "identb": identb,
                "identf": identf,
            }
        )
    return run_bass_kernel_spmd(nc, in_maps, core_ids=list(range(N_CORES)), **kw)


def combine_partials(partials_list):
    q = s2 = m = 0.0
    for p in partials_list:
        p = np.asarray(p, dtype=np.float64)
        q += p[0].sum()
        s2 += p[1].sum()
        m += p[2].sum()
    var_mean = (q - s2 / L) / (L - 1) / B
    return var_mean + 0.5 * m / B


def kernel(pred_ratios, demands, tunnel_to_link, link_capacities):
    mask, identb, identf = make_constants(tunnel_to_link, link_capacities)
    nc = build_kernel()
    res = run_cores(nc, pred_ratios, demands, mask, identb, identf)
    loss = combine_partials([r["partials"] for r in res.results])
    return np.array(loss, dtype=np.float32)
